# revision 35
# baseline (speedup 1.0000x reference)
"""
Trainium2 Bass kernel for nn_MultiHeadAttention_74586402062628.

Data-parallel across 8 NeuronCores: one batch element per core.

v2: bf16 data path (fp32 PSUM accumulation everywhere), interleaved
emission so the PE stays continuously busy (TRN2 DVFS only reaches the
2.4 GHz peak after ~3us of uninterrupted PE work), and a cheap softmax
denominator path.

Per-core formulation (B=8, S=1000, E=1024, H=16, D=64):
  - x [S,E] arrives bf16, transposed on-chip (PE transpose, 4 blocks per
    PSUM tile) to xT [E,S] bf16.
  - Q,K projections produce qT,kT [H*D, S] bf16 (head h lives in 128-row
    tile h//2 at partition base (h%2)*64). V produces v in natural
    layout [S, H*D] bf16, scattered per-head, with an extra "ones"
    column so the AV matmul emits the softmax denominator as row 64.
  - Attention is computed transposed: scoresT[t,s] = k_t . q_s. exp()
    without max-subtraction (logits are small), causal masking as a 0/1
    band multiply (only the 128-wide diagonal band is ever computed:
    score tiles start at column off = t0-s0, which bf16 matmuls allow at
    full rate for any width).
  - Softmax denominators are reciprocated with the single-instruction
    DVE approx (reciprocal_approx_fast, ~18 bits), broadcast across
    partitions with a K=1 ones matmul, and multiplied into zT -> yT.
  - Output projection accumulates + bp via a K=1 ones matmul; results
    are staged bf16 and GELU'd in one batch at the end (Exp<->Gelu
    activation-table swaps cost 1.3us each, so Gelu runs exactly once).
  - bv is folded into an effective output bias bpe = bp + bv @ wp (valid
    because softmax rows sum to 1); bq/bk are added on PSUM eviction.

Emission interleaves QKV / attention / output-projection work so each
engine queue (PE, ACT, DVE) always has independent work while another
engine resolves a dependency.
"""

import math
import os
import sys

for _p in ("/opt/trn_rl_repo", "/opt/pypackages"):
    if _p not in sys.path:
        sys.path.insert(0, _p)

import numpy as np

B, S, E, H, D = 8, 1000, 1024, 16, 64
P = 128
NB = 8                      # 128-row blocks covering S (last is partial)
LAST = S - (NB - 1) * P     # 104
KT = 8                      # 128-row contraction tiles covering E
ST = ((0, 512), (512, 488))     # s tiles (start, width) covering S
FT = ((0, 512), (512, 512))     # f/n tiles covering E
SCALE = 1.0 / math.sqrt(S)
NCORES = 8

# BASSMHA_NO_GELU=1: replace final GELU with Identity (CoreSim lacks Gelu)
_NO_GELU = os.environ.get("BASSMHA_NO_GELU", "0") == "1"
# BASSMHA_DEBUG=1: add intermediate dumps (yT, qT, kT, rp) as outputs
_DEBUG = os.environ.get("BASSMHA_DEBUG", "0") == "1"

_CACHE = {}


def _build_nc():
    import dataclasses as _dc

    from concourse import bacc
    import concourse.mybir as mybir
    from concourse import tile
    from concourse.masks import make_identity

    dt = mybir.dt
    f32 = dt.float32
    f32r = dt.float32r
    bf = dt.bfloat16
    AF = mybir.ActivationFunctionType
    Alu = mybir.AluOpType

    nc = bacc.Bacc("TRN2", debug=False, target_bir_lowering=False,
                   num_devices=NCORES)

    x_d = nc.declare_dram_parameter("xb", [S, E], bf, isOutput=False)
    wq_d = nc.declare_dram_parameter("wq2", [E, E], bf, isOutput=False)
    wk_d = nc.declare_dram_parameter("wk2", [E, E], bf, isOutput=False)
    wv_d = nc.declare_dram_parameter("wv2", [E, E], bf, isOutput=False)
    wp_d = nc.declare_dram_parameter("wp2", [E, E], bf, isOutput=False)
    bq_d = nc.declare_dram_parameter("bqt", [P, KT], f32, isOutput=False)
    bk_d = nc.declare_dram_parameter("bkt", [P, KT], f32, isOutput=False)
    bp_d = nc.declare_dram_parameter("bpe", [1, E], bf, isOutput=False)
    out_d = nc.declare_dram_parameter("out", [S, E], f32, isOutput=True)
    if _DEBUG:
        dbg_yT = nc.declare_dram_parameter("dbg_yT", [P, KT, S], bf,
                                           isOutput=True)
        dbg_qT = nc.declare_dram_parameter("dbg_qT", [P, KT, S], bf,
                                           isOutput=True)
        dbg_kT = nc.declare_dram_parameter("dbg_kT", [P, KT, S], bf,
                                           isOutput=True)
        dbg_rp = nc.declare_dram_parameter("dbg_rp", [16, 1024], f32,
                                           isOutput=True)
        dbg_dn = nc.declare_dram_parameter("dbg_dn", [16, 1024], f32,
                                           isOutput=True)

    def g2(ap):
        return ap.rearrange("p (g c) -> p g c", g=2)

    def bcast2(ap):
        return _dc.replace(ap, ap=[ap.ap[0], [0, 2], ap.ap[1]])

    with tile.TileContext(nc) as tc:
        with (
            tc.tile_pool(name="const", bufs=1) as constp,
            tc.tile_pool(name="persist", bufs=1) as persist,
            tc.tile_pool(name="xload", bufs=1) as xload,
            tc.tile_pool(name="wqk", bufs=12) as wqkp,
            tc.tile_pool(name="wv", bufs=8) as wvp,
            tc.tile_pool(name="wp", bufs=16) as wpp,
            tc.tile_pool(name="exp", bufs=5) as expp,
            tc.tile_pool(name="zraw", bufs=10) as zrawp,
            tc.tile_pool(name="dnc", bufs=2) as dncp,
            tc.tile_pool(name="dng", bufs=2) as dngp,
            tc.tile_pool(name="rpg", bufs=1) as rpgp,
            tc.tile_pool(name="zt", bufs=2) as ztp,
            tc.tile_pool(name="outp", bufs=4) as outp,
            # PSUM budget (8 banks): ps512 2x1 + sp 2x2 + zp 1x2 = 8
            tc.tile_pool(name="ps512", bufs=2, space="PSUM") as ps512,
            tc.tile_pool(name="sp", bufs=2, space="PSUM") as spsum,
            tc.tile_pool(name="zp", bufs=1, space="PSUM") as zpsum,
        ):
            # ---------------- constants ----------------
            ident = constp.tile([P, P], bf)
            make_identity(nc, ident[:])
            # msk[r, c] = 1.0 iff c >= r (keep); the causal band mask
            msk = constp.tile([P, P], bf)
            nc.gpsimd.memset(msk[:], 1.0)
            nc.gpsimd.affine_select(
                out=msk[:], in_=msk[:],
                compare_op=Alu.is_ge, fill=0.0,
                base=0, channel_multiplier=-1, pattern=[[1, P]],
            )
            # selc[k, c] = 1.0 iff c//64 == k: block-diagonal row-selector.
            # lhsT selc[0:8, hp*64:(hp+1)*64] broadcasts rp_g row hp onto
            # partitions 0:64 (PE matmul base-partition must be 0/32/64,
            # so single-row lhsT slices at partition hp are not allowed).
            selc = constp.tile([P, 512], bf)
            nc.gpsimd.memset(selc[:], 1.0)
            nc.gpsimd.affine_select(
                out=selc[:], in_=selc[:],
                compare_op=Alu.is_ge, fill=0.0,
                base=0, channel_multiplier=-64, pattern=[[1, 512]],
            )
            nc.gpsimd.affine_select(
                out=selc[:], in_=selc[:],
                compare_op=Alu.is_ge, fill=0.0,
                base=63, channel_multiplier=64, pattern=[[-1, 512]],
            )
            ones_b = constp.tile([P, P], bf)     # for bf16 K=1 bias rows
            nc.gpsimd.memset(ones_b[:], 1.0)

            bq_sb = constp.tile([P, KT], f32)
            nc.sync.dma_start(bq_sb[:], bq_d[:, :])
            bk_sb = constp.tile([P, KT], f32)
            nc.sync.dma_start(bk_sb[:], bk_d[:, :])
            bp_sb = constp.tile([1, E], bf)
            nc.sync.dma_start(bp_sb[:], bp_d[:, :])

            # ---------------- persistent activations ----------------
            qT = persist.tile([P, KT, S], bf)        # [hd, m, s]
            kT = persist.tile([P, KT, S], bf)
            # v stationary slabs per (pair, t_block), both parities:
            # [v(64) | ones] -> psum rows 0:64 = zT, row 64 = denom.
            v_e = persist.tile([P, H // 2, NB, 65], bf)
            v_o = persist.tile([P, H // 2, NB, 65], bf)
            yT = persist.tile([P, KT, S], bf)        # normalized z, stacked
            xT = persist.tile([P, KT, S], bf)
            ostage = persist.tile([P, NB, E], bf)    # pre-GELU staging

            # v slab init: zero the tb7 padding rows first (t >= 1000 must
            # not contribute to z or the denominator), then set the ones
            # ("denominator") column only on valid rows.
            nc.vector.memset(v_e[96:P, :, NB - 1, :], 0.0)
            nc.vector.memset(v_o[96:P, :, NB - 1, :], 0.0)
            nc.vector.memset(v_e[:, :, 0:NB - 1, 64:65], 1.0)
            nc.vector.memset(v_o[:, :, 0:NB - 1, 64:65], 1.0)
            nc.vector.memset(v_e[0:LAST, :, NB - 1, 64:65], 1.0)
            nc.vector.memset(v_o[0:LAST, :, NB - 1, 64:65], 1.0)

            # ---------------- phase 1: x -> xT ----------------
            xts = []
            for tb in range(NB):
                rows = LAST if tb == NB - 1 else P
                xt = xload.tile([P, E], bf, tag=f"x{tb}", name=f"xt{tb}")
                nc.sync.dma_start(xt[0:rows, :], x_d[tb * P:tb * P + rows, :])
                xts.append(xt)
            for eb in range(KT):
                for gph in range(2):
                    wid = 512 if gph == 0 else 384 + LAST
                    tp = ps512.tile([P, 512], bf, tag="b",
                                    name=f"tp{eb}_{gph}")
                    for j in range(4):
                        tb = 4 * gph + j
                        rows = LAST if tb == NB - 1 else P
                        nc.tensor.transpose(
                            tp[0:P, j * P:j * P + rows],
                            xts[tb][0:rows, eb * P:(eb + 1) * P],
                            ident[0:rows, 0:rows])
                    nc.scalar.activation(
                        xT[:, eb, gph * 512:gph * 512 + wid],
                        tp[0:P, 0:wid], AF.Copy)

            # ---------------- QKV emission helpers ----------------
            def emit_qk1(wd, dst, bias, mg):
                wts = []
                for k in range(KT):
                    wt = wqkp.tile([P, 2 * P], bf, tag="wqk",
                                   name=f"w{mg}_{k}")
                    nc.sync.dma_start(
                        wt[:], wd[k * P:(k + 1) * P,
                                  mg * 2 * P:(mg + 1) * 2 * P])
                    wts.append(wt)
                for mi in range(2):
                    m = 2 * mg + mi
                    for (s0, W) in ST:
                        ps = ps512.tile([P, 512], f32, tag="b",
                                        name=f"qkps{m}_{s0}")
                        for k in range(KT):
                            nc.tensor.matmul(
                                ps[0:P, 0:W],
                                wts[k][:, mi * P:(mi + 1) * P],
                                xT[:, k, s0:s0 + W],
                                start=(k == 0), stop=(k == KT - 1))
                        with nc.allow_low_precision(reason="bf16 evict"):
                            nc.vector.tensor_scalar_add(
                                dst[:, m, s0:s0 + W], ps[0:P, 0:W],
                                bias[:, m:m + 1])

            def emit_qk(mg):
                emit_qk1(wq_d, qT, bq_sb, mg)
                emit_qk1(wk_d, kT, bk_sb, mg)

            def emit_v(nt):
                n0, Wn = FT[nt]
                wvs = []
                for k in range(KT):
                    wv = wvp.tile([P, 512], bf, tag="wv", name=f"wv{nt}_{k}")
                    nc.sync.dma_start(
                        wv[0:P, 0:Wn], wv_d[k * P:(k + 1) * P, n0:n0 + Wn])
                    wvs.append(wv)
                for tb in range(NB):
                    rows = LAST if tb == NB - 1 else P
                    t0 = tb * P
                    ps = ps512.tile([P, 512], f32, tag="b",
                                    name=f"vps{nt}_{tb}")
                    for k in range(KT):
                        nc.tensor.matmul(
                            ps[0:rows, 0:Wn],
                            xT[:, k, t0:t0 + rows],
                            wvs[k][0:P, 0:Wn],
                            start=(k == 0), stop=(k == KT - 1))
                    src = ps[0:rows, 0:Wn].rearrange("p (h e) -> p h e", e=P)
                    hp0 = 4 * nt
                    with nc.allow_low_precision(reason="bf16 evict"):
                        nc.scalar.activation(
                            v_e[0:rows, hp0:hp0 + 4, tb, 0:64],
                            src[:, :, 0:64], AF.Copy)
                        nc.scalar.activation(
                            v_o[0:rows, hp0:hp0 + 4, tb, 0:64],
                            src[:, :, 64:128], AF.Copy)

            # ---------------- attention emission ----------------
            PIPE = 2
            # per-phase denominator gather tiles: row hp holds the f32
            # denominators (even head at [0:W], odd at [512:512+W])
            dn_gs = {}

            def emit_dng(sti):
                dn_g = dngp.tile([P, 1024], bf, tag="dng",
                                 name=f"dng{sti}")
                # unused columns must stay finite for the batched recip
                nc.vector.memset(dn_g[0:8, :], 1.0)
                dn_gs[sti] = dn_g

            def emit_attn(hp, sti):
                s0, W = ST[sti]
                n_tb = (s0 + W + P - 1) // P
                zp = zpsum.tile([P, 1024], f32, tag="zp",
                                name=f"zp{hp}_{sti}")
                exs = {}
                geom = {}
                for tb in range(n_tb):
                    rows = LAST if tb == NB - 1 else P
                    t0 = tb * P
                    off = max(0, t0 - s0)
                    geom[tb] = (rows, t0, off, W - off, t0 >= s0)
                for i in range(n_tb + PIPE):
                    if i < n_tb:
                        tb = i
                        rows, t0, off, N, has_diag = geom[tb]
                        sp = spsum.tile([P, 1024], f32, tag="sp",
                                        name=f"sp{hp}_{sti}_{tb}")
                        for par in range(2):
                            base = par * 64
                            nc.tensor.matmul(
                                sp[0:rows, 512 * par:512 * par + N],
                                kT[base:base + 64, hp, t0:t0 + rows],
                                qT[base:base + 64, hp, s0 + off:s0 + W],
                                start=True, stop=True)
                        ex = expp.tile([P, 1024], bf, tag="ex",
                                       name=f"ex{hp}_{sti}_{tb}")
                        exv, spv = g2(ex[:, :]), g2(sp[:, :])
                        if rows < P:
                            nc.vector.memset(exv[96:P, :, 0:N], 0.0)
                        with nc.allow_low_precision(reason="bf16 exp"):
                            nc.scalar.activation(
                                exv[0:rows, :, 0:N], spv[0:rows, :, 0:N],
                                AF.Exp, scale=SCALE)
                        if has_diag:
                            dw = min(P, N)
                            with nc.allow_low_precision(reason="bf16 mask"):
                                nc.vector.tensor_tensor(
                                    exv[0:rows, :, 0:dw],
                                    exv[0:rows, :, 0:dw],
                                    bcast2(msk[0:rows, 0:dw]),
                                    op=Alu.mult)
                        exs[tb] = ex
                    j = i - PIPE
                    if 0 <= j < n_tb:
                        rows, t0, off, N, has_diag = geom[j]
                        ex = exs.pop(j)
                        for par, vs in ((0, v_e), (1, v_o)):
                            nc.tensor.matmul(
                                zp[0:65, 512 * par + off:512 * par + W],
                                vs[:, hp, j, 0:65],
                                ex[0:P, 512 * par:512 * par + N],
                                start=(j == 0), stop=(j == n_tb - 1),
                                skip_group_check=True)
                # evict unnormalized z bf16 and the f32 denominator row;
                # normalization is batched per s-phase (emit_norm) so one
                # wide DVE reciprocal covers all 8 head-pairs.
                zpv = g2(zp[:, :])
                zraw = zrawp.tile([P, 1024], bf, tag="zr",
                                  name=f"zr{hp}_{sti}")
                with nc.allow_low_precision(reason="bf16 z evict"):
                    nc.vector.tensor_copy(
                        g2(zraw[:, :])[0:64, :, 0:W], zpv[0:64, :, 0:W])
                dnc = dncp.tile([P, 1024], bf, tag="dnc",
                                name=f"dnc{hp}_{sti}")
                with nc.allow_low_precision(reason="bf16 denom"):
                    nc.vector.tensor_copy(
                        g2(dnc[:, :])[64:65, :, 0:W], zpv[64:65, :, 0:W])
                # partition-move the denom row onto partition hp of dn_g
                nc.sync.dma_start(
                    g2(dn_gs[sti][:, :])[hp:hp + 1, :, 0:W],
                    g2(dnc[:, :])[64:65, :, 0:W])
                return zraw

            def emit_norm(sti, zraws):
                s0, W = ST[sti]
                dn_g = dn_gs[sti]
                rp_g = rpgp.tile([P, 1024], bf, tag="rpg",
                                 name=f"rpg{sti}")
                with nc.allow_low_precision(
                        reason="1/denom rounds to bf16 for the broadcast"):
                    nc.vector.reciprocal(rp_g[0:8, :], dn_g[0:8, :])
                if _DEBUG:
                    nc.sync.dma_start(dbg_dn[sti * 8:sti * 8 + 8, :],
                                      dn_g[0:8, :])
                for hp in range(8):
                    zraw = zraws[hp]
                    # one wide broadcast covers both parities (cols 0:512
                    # even, 512:1024 odd); psum borrowed from the sp pool
                    bc = spsum.tile([P, 1024], f32, tag="sp",
                                    name=f"bc{hp}_{sti}")
                    for par in range(2):
                        nc.tensor.matmul(
                            bc[0:64, 512 * par:512 * par + W],
                            selc[0:8, hp * 64:(hp + 1) * 64],
                            rp_g[0:8, 512 * par:512 * par + W],
                            start=True, stop=True)
                    with nc.allow_low_precision(reason="bf16 yT"):
                        nc.vector.tensor_tensor(
                            yT[0:64, hp, s0:s0 + W],
                            zraw[0:64, 0:W], bc[0:64, 0:W],
                            op=Alu.mult)
                        # odd heads partition-shift 0:64 -> 64:128 via
                        # SBUF->SBUF DMA
                        zt_o = ztp.tile([64, 512], bf, tag="zt",
                                        name=f"zt{hp}_{sti}")
                        nc.vector.tensor_tensor(
                            zt_o[0:64, 0:W],
                            zraw[0:64, 512:512 + W],
                            bc[0:64, 512:512 + W],
                            op=Alu.mult)
                        nc.sync.dma_start(
                            yT[64:P, hp, s0:s0 + W], zt_o[0:64, 0:W])

            # ---------------- output projection ----------------
            def emit_wp_loads():
                wps = {}
                for fi, (f0, Fw) in enumerate(FT):
                    for k in range(KT):
                        w = wpp.tile([P, 512], bf, tag="wp",
                                     name=f"wp{fi}_{k}")
                        nc.sync.dma_start(
                            w[0:P, 0:Fw], wp_d[k * P:(k + 1) * P, f0:f0 + Fw])
                        wps[(fi, k)] = w
                return wps

            def emit_p4(wps, fi, sb):
                f0, Fw = FT[fi]
                rows = LAST if sb == NB - 1 else P
                r0 = sb * P
                ps = ps512.tile([P, 512], f32, tag="b", name=f"p4{fi}_{sb}")
                for k in range(KT):
                    nc.tensor.matmul(
                        ps[0:rows, 0:Fw],
                        yT[:, k, r0:r0 + rows],
                        wps[(fi, k)][0:P, 0:Fw],
                        start=(k == 0), stop=False)
                # + bias row via K=1 ones matmul
                nc.tensor.matmul(
                    ps[0:rows, 0:Fw],
                    ones_b[0:1, 0:rows],
                    bp_sb[0:1, f0:f0 + Fw],
                    start=False, stop=True)
                with nc.allow_low_precision(reason="bf16 stage"):
                    nc.scalar.activation(
                        ostage[0:rows, sb, f0:f0 + Fw],
                        ps[0:rows, 0:Fw], AF.Copy)

            def emit_out(fi, sb):
                f0, Fw = FT[fi]
                rows = LAST if sb == NB - 1 else P
                r0 = sb * P
                ot = outp.tile([P, 512], f32, tag="ot", name=f"ot{fi}_{sb}")
                act = AF.Identity if _NO_GELU else AF.Gelu
                nc.scalar.activation(
                    ot[0:rows, 0:Fw], ostage[0:rows, sb, f0:f0 + Fw], act)
                nc.sync.dma_start(out_d[r0:r0 + rows, f0:f0 + Fw],
                                  ot[0:rows, 0:Fw])

            # ---------------- interleaved schedule ----------------
            # QKV chunks keep the PE fed while earlier head-pairs run
            # their (ACT/DVE-heavy) attention; once st0 is complete for
            # all head-pairs, output-projection chunks fill the PE during
            # st1 attention. GELU runs once at the very end.
            z0, z1 = {}, {}
            emit_dng(0)
            emit_qk(0)
            emit_v(0)
            emit_qk(1)
            z0[0] = emit_attn(0, 0)
            emit_qk(2)
            z0[1] = emit_attn(1, 0)
            emit_qk(3)
            z0[2] = emit_attn(2, 0)
            emit_v(1)
            z0[3] = emit_attn(3, 0)
            z0[4] = emit_attn(4, 0)
            z0[5] = emit_attn(5, 0)
            z0[6] = emit_attn(6, 0)
            z0[7] = emit_attn(7, 0)
            wps = emit_wp_loads()
            emit_dng(1)
            # A(0,1)+A(1,1) keep the PE busy while norm(0) runs on DVE
            z1[0] = emit_attn(0, 1)
            z1[1] = emit_attn(1, 1)
            emit_norm(0, z0)
            # p4 for s-blocks 0-3 only needs st0 results; sb 4-7 need
            # every head-pair's st1, so they trail the last attention.
            emit_p4(wps, 0, 0)
            emit_p4(wps, 1, 0)
            z1[2] = emit_attn(2, 1)
            emit_p4(wps, 0, 1)
            z1[3] = emit_attn(3, 1)
            emit_p4(wps, 1, 1)
            z1[4] = emit_attn(4, 1)
            emit_p4(wps, 0, 2)
            z1[5] = emit_attn(5, 1)
            emit_p4(wps, 1, 2)
            z1[6] = emit_attn(6, 1)
            emit_p4(wps, 0, 3)
            z1[7] = emit_attn(7, 1)
            emit_p4(wps, 1, 3)
            emit_norm(1, z1)
            # sb 4-7 stores can only become ready after the last attention
            # EXP (their p4 chunks follow norm(1)), so the scheduler cannot
            # hoist their GELUs into the Exp stretch: emit normally and they
            # pipeline with the trailing p4 chunks.
            for sb in range(4, NB):
                emit_p4(wps, 0, sb)
                emit_out(0, sb)
                emit_p4(wps, 1, sb)
                emit_out(1, sb)
            # sb 0-3 staging is ready mid-attention; force these GELUs to
            # the end so no Exp<->Gelu activation-table swap (1.3us each)
            # lands inside the attention stretch.
            with tc.tile_wait_until(0.5):
                for sb in range(4):
                    emit_out(0, sb)
                    emit_out(1, sb)

            if _DEBUG:
                nc.sync.dma_start(dbg_yT[:, :, :], yT[:, :, :])
                nc.sync.dma_start(dbg_qT[:, :, :], qT[:, :, :])
                nc.sync.dma_start(dbg_kT[:, :, :], kT[:, :, :])

    nc.compile()
    return nc


def get_nc():
    if "nc" not in _CACHE:
        _CACHE["nc"] = _build_nc()
    return _CACHE["nc"]


def make_in_maps(inputs):
    import ml_dtypes
    bfnp = ml_dtypes.bfloat16

    x = np.asarray(inputs["x"], np.float32)
    wq = np.asarray(inputs["wq"], np.float32)
    wk = np.asarray(inputs["wk"], np.float32)
    wv = np.asarray(inputs["wv"], np.float32)
    wp = np.asarray(inputs["wp"], np.float32)
    bq = np.asarray(inputs["bq"], np.float32)
    bk = np.asarray(inputs["bk"], np.float32)
    bv = np.asarray(inputs["bv"], np.float32)
    bp = np.asarray(inputs["bp"], np.float32)

    # [H, E, D] -> [E, H*D] (concat head outputs along columns)
    wq2 = np.ascontiguousarray(
        wq.transpose(1, 0, 2).reshape(E, E).astype(bfnp))
    wk2 = np.ascontiguousarray(
        wk.transpose(1, 0, 2).reshape(E, E).astype(bfnp))
    wv2 = np.ascontiguousarray(
        wv.transpose(1, 0, 2).reshape(E, E).astype(bfnp))
    wp2 = np.ascontiguousarray(wp.astype(bfnp))
    # per-partition bias layout: bqt[p, m] = bq_flat[m*128 + p]
    bqt = np.ascontiguousarray(bq.reshape(-1).reshape(KT, P).T)
    bkt = np.ascontiguousarray(bk.reshape(-1).reshape(KT, P).T)
    # fold bv into output bias: y = z + bv  =>  out += bv @ wp
    bpe = (bp.astype(np.float64)
           + bv.reshape(-1).astype(np.float64) @ wp.astype(np.float64))
    bpe = np.ascontiguousarray(
        bpe.astype(np.float32).reshape(1, E).astype(bfnp))

    shared = {"wq2": wq2, "wk2": wk2, "wv2": wv2, "wp2": wp2,
              "bqt": bqt, "bkt": bkt, "bpe": bpe}
    return [dict(shared, xb=np.ascontiguousarray(x[b].astype(bfnp)))
            for b in range(B)]


def run(inputs, trace=False):
    from concourse.bass_utils import run_bass_kernel_spmd
    nc = get_nc()
    in_maps = make_in_maps(inputs)
    res = run_bass_kernel_spmd(nc, in_maps, list(range(NCORES)), trace=trace)
    out = np.stack([np.asarray(res.results[i]["out"]) for i in range(NCORES)])
    return out.astype(np.float32), res


def kernel(**inputs):
    out, _ = run(inputs, trace=False)
    return out


# revision 36
# speedup vs baseline: 1.2027x; 1.2027x over previous
"""
Trainium2 Bass kernel for nn_MultiHeadAttention_74586402062628.

Data-parallel across 8 NeuronCores: one batch element per core.

v2: bf16 data path (fp32 PSUM accumulation everywhere), interleaved
emission so the PE stays continuously busy (TRN2 DVFS only reaches the
2.4 GHz peak after ~3us of uninterrupted PE work), and a cheap softmax
denominator path.

Per-core formulation (B=8, S=1000, E=1024, H=16, D=64):
  - x [S,E] arrives bf16, transposed on-chip (PE transpose, 4 blocks per
    PSUM tile) to xT [E,S] bf16.
  - Q,K projections produce qT,kT [H*D, S] bf16 (head h lives in 128-row
    tile h//2 at partition base (h%2)*64). V produces v in natural
    layout [S, H*D] bf16, scattered per-head, with an extra "ones"
    column so the AV matmul emits the softmax denominator as row 64.
  - Attention is computed transposed: scoresT[t,s] = k_t . q_s. exp()
    without max-subtraction (logits are small), causal masking as a 0/1
    band multiply (only the 128-wide diagonal band is ever computed:
    score tiles start at column off = t0-s0, which bf16 matmuls allow at
    full rate for any width).
  - Softmax denominators are reciprocated with the single-instruction
    DVE approx (reciprocal_approx_fast, ~18 bits), broadcast across
    partitions with a K=1 ones matmul, and multiplied into zT -> yT.
  - Output projection accumulates + bp via a K=1 ones matmul; results
    are staged bf16 and GELU'd in one batch at the end (Exp<->Gelu
    activation-table swaps cost 1.3us each, so Gelu runs exactly once).
  - bv is folded into an effective output bias bpe = bp + bv @ wp (valid
    because softmax rows sum to 1); bq/bk are added on PSUM eviction.

Emission interleaves QKV / attention / output-projection work so each
engine queue (PE, ACT, DVE) always has independent work while another
engine resolves a dependency.
"""

import math
import os
import sys

for _p in ("/opt/trn_rl_repo", "/opt/pypackages"):
    if _p not in sys.path:
        sys.path.insert(0, _p)

import numpy as np

B, S, E, H, D = 8, 1000, 1024, 16, 64
P = 128
NB = 8                      # 128-row blocks covering S (last is partial)
LAST = S - (NB - 1) * P     # 104
KT = 8                      # 128-row contraction tiles covering E
ST = ((0, 512), (512, 488))     # s tiles (start, width) covering S
FT = ((0, 512), (512, 512))     # f/n tiles covering E
SCALE = 1.0 / math.sqrt(S)
NCORES = 8

# BASSMHA_NO_GELU=1: replace final GELU with Identity (CoreSim lacks Gelu)
_NO_GELU = os.environ.get("BASSMHA_NO_GELU", "0") == "1"
# BASSMHA_DEBUG=1: add intermediate dumps (yT, qT, kT, rp) as outputs
_DEBUG = os.environ.get("BASSMHA_DEBUG", "0") == "1"

_CACHE = {}


def _build_nc():
    import dataclasses as _dc

    from concourse import bacc
    import concourse.mybir as mybir
    from concourse import tile
    from concourse.masks import make_identity

    dt = mybir.dt
    f32 = dt.float32
    f32r = dt.float32r
    bf = dt.bfloat16
    AF = mybir.ActivationFunctionType
    Alu = mybir.AluOpType

    nc = bacc.Bacc("TRN2", debug=False, target_bir_lowering=False,
                   num_devices=NCORES)

    x_d = nc.declare_dram_parameter("xb", [S, E], bf, isOutput=False)
    wq_d = nc.declare_dram_parameter("wq2", [E, E], bf, isOutput=False)
    wk_d = nc.declare_dram_parameter("wk2", [E, E], bf, isOutput=False)
    wv_d = nc.declare_dram_parameter("wv2", [E, E], bf, isOutput=False)
    wp_d = nc.declare_dram_parameter("wp2", [E, E], bf, isOutput=False)
    bq_d = nc.declare_dram_parameter("bqt", [P, KT], f32, isOutput=False)
    bk_d = nc.declare_dram_parameter("bkt", [P, KT], f32, isOutput=False)
    bp_d = nc.declare_dram_parameter("bpe", [1, E], bf, isOutput=False)
    out_d = nc.declare_dram_parameter("out", [S, E], f32, isOutput=True)
    if _DEBUG:
        dbg_yT = nc.declare_dram_parameter("dbg_yT", [P, KT, S], bf,
                                           isOutput=True)
        dbg_qT = nc.declare_dram_parameter("dbg_qT", [P, KT, S], bf,
                                           isOutput=True)
        dbg_kT = nc.declare_dram_parameter("dbg_kT", [P, KT, S], bf,
                                           isOutput=True)
        dbg_rp = nc.declare_dram_parameter("dbg_rp", [16, 1024], f32,
                                           isOutput=True)
        dbg_dn = nc.declare_dram_parameter("dbg_dn", [16, 1024], f32,
                                           isOutput=True)

    def g2(ap):
        return ap.rearrange("p (g c) -> p g c", g=2)

    def bcast2(ap):
        return _dc.replace(ap, ap=[ap.ap[0], [0, 2], ap.ap[1]])

    with tile.TileContext(nc) as tc:
        with (
            tc.tile_pool(name="const", bufs=1) as constp,
            tc.tile_pool(name="persist", bufs=1) as persist,
            tc.tile_pool(name="xload", bufs=1) as xload,
            tc.tile_pool(name="wqk", bufs=12) as wqkp,
            tc.tile_pool(name="wv", bufs=8) as wvp,
            tc.tile_pool(name="wp", bufs=16) as wpp,
            tc.tile_pool(name="exp", bufs=5) as expp,
            tc.tile_pool(name="zraw", bufs=10) as zrawp,
            tc.tile_pool(name="dnc", bufs=2) as dncp,
            tc.tile_pool(name="dng", bufs=2) as dngp,
            tc.tile_pool(name="rpg", bufs=1) as rpgp,
            tc.tile_pool(name="zt", bufs=2) as ztp,
            tc.tile_pool(name="outp", bufs=4) as outp,
            # PSUM budget (8 banks): ps512 2x1 + sp 2x2 + zp 1x2 = 8
            tc.tile_pool(name="ps512", bufs=2, space="PSUM") as ps512,
            tc.tile_pool(name="sp", bufs=2, space="PSUM") as spsum,
            tc.tile_pool(name="zp", bufs=1, space="PSUM") as zpsum,
        ):
            # ---------------- constants ----------------
            ident = constp.tile([P, P], bf)
            make_identity(nc, ident[:])
            # msk[r, c] = 1.0 iff c >= r (keep); the causal band mask
            msk = constp.tile([P, P], bf)
            nc.gpsimd.memset(msk[:], 1.0)
            nc.gpsimd.affine_select(
                out=msk[:], in_=msk[:],
                compare_op=Alu.is_ge, fill=0.0,
                base=0, channel_multiplier=-1, pattern=[[1, P]],
            )
            # selc[k, c] = 1.0 iff c//64 == k: block-diagonal row-selector.
            # lhsT selc[0:8, hp*64:(hp+1)*64] broadcasts rp_g row hp onto
            # partitions 0:64 (PE matmul base-partition must be 0/32/64,
            # so single-row lhsT slices at partition hp are not allowed).
            selc = constp.tile([P, 512], bf)
            nc.gpsimd.memset(selc[:], 1.0)
            nc.gpsimd.affine_select(
                out=selc[:], in_=selc[:],
                compare_op=Alu.is_ge, fill=0.0,
                base=0, channel_multiplier=-64, pattern=[[1, 512]],
            )
            nc.gpsimd.affine_select(
                out=selc[:], in_=selc[:],
                compare_op=Alu.is_ge, fill=0.0,
                base=63, channel_multiplier=64, pattern=[[-1, 512]],
            )
            ones_b = constp.tile([P, P], bf)     # for bf16 K=1 bias rows
            nc.gpsimd.memset(ones_b[:], 1.0)

            bq_sb = constp.tile([P, KT], f32)
            nc.sync.dma_start(bq_sb[:], bq_d[:, :])
            bk_sb = constp.tile([P, KT], f32)
            nc.sync.dma_start(bk_sb[:], bk_d[:, :])
            bp_sb = constp.tile([1, E], bf)
            nc.sync.dma_start(bp_sb[:], bp_d[:, :])

            # ---------------- persistent activations ----------------
            qT = persist.tile([P, KT, S], bf)        # [hd, m, s]
            kT = persist.tile([P, KT, S], bf)
            # v stationary slabs per (pair, t_block), both parities:
            # [v(64) | ones] -> psum rows 0:64 = zT, row 64 = denom.
            v_e = persist.tile([P, H // 2, NB, 65], bf)
            v_o = persist.tile([P, H // 2, NB, 65], bf)
            yT = persist.tile([P, KT, S], bf)        # normalized z, stacked
            xT = persist.tile([P, KT, S], bf)
            ostage = persist.tile([P, NB, E], bf)    # pre-GELU staging

            # v slab init: zero the tb7 padding rows first (t >= 1000 must
            # not contribute to z or the denominator), then set the ones
            # ("denominator") column only on valid rows.
            nc.vector.memset(v_e[96:P, :, NB - 1, :], 0.0)
            nc.vector.memset(v_o[96:P, :, NB - 1, :], 0.0)
            nc.vector.memset(v_e[:, :, 0:NB - 1, 64:65], 1.0)
            nc.vector.memset(v_o[:, :, 0:NB - 1, 64:65], 1.0)
            nc.vector.memset(v_e[0:LAST, :, NB - 1, 64:65], 1.0)
            nc.vector.memset(v_o[0:LAST, :, NB - 1, 64:65], 1.0)

            # ---------------- phase 1: x -> xT ----------------
            xts = []
            for tb in range(NB):
                rows = LAST if tb == NB - 1 else P
                xt = xload.tile([P, E], bf, tag=f"x{tb}", name=f"xt{tb}")
                nc.sync.dma_start(xt[0:rows, :], x_d[tb * P:tb * P + rows, :])
                xts.append(xt)
            for eb in range(KT):
                for gph in range(2):
                    wid = 512 if gph == 0 else 384 + LAST
                    tp = ps512.tile([P, 512], bf, tag="b",
                                    name=f"tp{eb}_{gph}")
                    for j in range(4):
                        tb = 4 * gph + j
                        rows = LAST if tb == NB - 1 else P
                        nc.tensor.transpose(
                            tp[0:P, j * P:j * P + rows],
                            xts[tb][0:rows, eb * P:(eb + 1) * P],
                            ident[0:rows, 0:rows])
                    nc.scalar.activation(
                        xT[:, eb, gph * 512:gph * 512 + wid],
                        tp[0:P, 0:wid], AF.Copy)

            # ---------------- QKV emission helpers ----------------
            def emit_qk1(wd, dst, bias, mg):
                wts = []
                for k in range(KT):
                    wt = wqkp.tile([P, 2 * P], bf, tag="wqk",
                                   name=f"w{mg}_{k}")
                    nc.sync.dma_start(
                        wt[:], wd[k * P:(k + 1) * P,
                                  mg * 2 * P:(mg + 1) * 2 * P])
                    wts.append(wt)
                for mi in range(2):
                    m = 2 * mg + mi
                    for (s0, W) in ST:
                        ps = ps512.tile([P, 512], f32, tag="b",
                                        name=f"qkps{m}_{s0}")
                        for k in range(KT):
                            nc.tensor.matmul(
                                ps[0:P, 0:W],
                                wts[k][:, mi * P:(mi + 1) * P],
                                xT[:, k, s0:s0 + W],
                                start=(k == 0), stop=(k == KT - 1))
                        with nc.allow_low_precision(reason="bf16 evict"):
                            nc.vector.tensor_scalar_add(
                                dst[:, m, s0:s0 + W], ps[0:P, 0:W],
                                bias[:, m:m + 1])

            def emit_qk(mg):
                emit_qk1(wq_d, qT, bq_sb, mg)
                emit_qk1(wk_d, kT, bk_sb, mg)

            def emit_v(nt):
                n0, Wn = FT[nt]
                wvs = []
                for k in range(KT):
                    wv = wvp.tile([P, 512], bf, tag="wv", name=f"wv{nt}_{k}")
                    nc.sync.dma_start(
                        wv[0:P, 0:Wn], wv_d[k * P:(k + 1) * P, n0:n0 + Wn])
                    wvs.append(wv)
                for tb in range(NB):
                    rows = LAST if tb == NB - 1 else P
                    t0 = tb * P
                    ps = ps512.tile([P, 512], f32, tag="b",
                                    name=f"vps{nt}_{tb}")
                    for k in range(KT):
                        nc.tensor.matmul(
                            ps[0:rows, 0:Wn],
                            xT[:, k, t0:t0 + rows],
                            wvs[k][0:P, 0:Wn],
                            start=(k == 0), stop=(k == KT - 1))
                    src = ps[0:rows, 0:Wn].rearrange("p (h e) -> p h e", e=P)
                    hp0 = 4 * nt
                    with nc.allow_low_precision(reason="bf16 evict"):
                        nc.scalar.activation(
                            v_e[0:rows, hp0:hp0 + 4, tb, 0:64],
                            src[:, :, 0:64], AF.Copy)
                        nc.scalar.activation(
                            v_o[0:rows, hp0:hp0 + 4, tb, 0:64],
                            src[:, :, 64:128], AF.Copy)

            # ---------------- attention emission ----------------
            PIPE = 2
            # per-phase denominator gather tiles: row hp holds the f32
            # denominators (even head at [0:W], odd at [512:512+W])
            dn_gs = {}

            def emit_dng(sti):
                dn_g = dngp.tile([P, 1024], bf, tag="dng",
                                 name=f"dng{sti}")
                # unused columns must stay finite for the batched recip
                nc.vector.memset(dn_g[0:8, :], 1.0)
                dn_gs[sti] = dn_g

            def emit_attn(hp, sti):
                s0, W = ST[sti]
                n_tb = (s0 + W + P - 1) // P
                zp = zpsum.tile([P, 1024], f32, tag="zp",
                                name=f"zp{hp}_{sti}")
                exs = {}
                geom = {}
                for tb in range(n_tb):
                    rows = LAST if tb == NB - 1 else P
                    t0 = tb * P
                    off = max(0, t0 - s0)
                    geom[tb] = (rows, t0, off, W - off, t0 >= s0)
                for i in range(n_tb + PIPE):
                    if i < n_tb:
                        tb = i
                        rows, t0, off, N, has_diag = geom[tb]
                        sp = spsum.tile([P, 1024], f32, tag="sp",
                                        name=f"sp{hp}_{sti}_{tb}")
                        for par in range(2):
                            base = par * 64
                            nc.tensor.matmul(
                                sp[0:rows, 512 * par:512 * par + N],
                                kT[base:base + 64, hp, t0:t0 + rows],
                                qT[base:base + 64, hp, s0 + off:s0 + W],
                                start=True, stop=True)
                        ex = expp.tile([P, 1024], bf, tag="ex",
                                       name=f"ex{hp}_{sti}_{tb}")
                        exv, spv = g2(ex[:, :]), g2(sp[:, :])
                        if rows < P:
                            nc.vector.memset(exv[96:P, :, 0:N], 0.0)
                        with nc.allow_low_precision(reason="bf16 exp"):
                            nc.scalar.activation(
                                exv[0:rows, :, 0:N], spv[0:rows, :, 0:N],
                                AF.Exp, scale=SCALE)
                        if has_diag:
                            dw = min(P, N)
                            with nc.allow_low_precision(reason="bf16 mask"):
                                nc.vector.tensor_tensor(
                                    exv[0:rows, :, 0:dw],
                                    exv[0:rows, :, 0:dw],
                                    bcast2(msk[0:rows, 0:dw]),
                                    op=Alu.mult)
                        exs[tb] = ex
                    j = i - PIPE
                    if 0 <= j < n_tb:
                        rows, t0, off, N, has_diag = geom[j]
                        ex = exs.pop(j)
                        for par, vs in ((0, v_e), (1, v_o)):
                            nc.tensor.matmul(
                                zp[0:65, 512 * par + off:512 * par + W],
                                vs[:, hp, j, 0:65],
                                ex[0:P, 512 * par:512 * par + N],
                                start=(j == 0), stop=(j == n_tb - 1),
                                skip_group_check=True)
                # evict unnormalized z bf16 and the f32 denominator row;
                # normalization is batched per s-phase (emit_norm) so one
                # wide DVE reciprocal covers all 8 head-pairs.
                zpv = g2(zp[:, :])
                zraw = zrawp.tile([P, 1024], bf, tag="zr",
                                  name=f"zr{hp}_{sti}")
                with nc.allow_low_precision(reason="bf16 z evict"):
                    nc.vector.tensor_copy(
                        g2(zraw[:, :])[0:64, :, 0:W], zpv[0:64, :, 0:W])
                dnc = dncp.tile([P, 1024], bf, tag="dnc",
                                name=f"dnc{hp}_{sti}")
                with nc.allow_low_precision(reason="bf16 denom"):
                    nc.vector.tensor_copy(
                        g2(dnc[:, :])[64:65, :, 0:W], zpv[64:65, :, 0:W])
                # partition-move the denom row onto partition hp of dn_g
                nc.sync.dma_start(
                    g2(dn_gs[sti][:, :])[hp:hp + 1, :, 0:W],
                    g2(dnc[:, :])[64:65, :, 0:W])
                return zraw

            def emit_norm(sti, zraws):
                s0, W = ST[sti]
                dn_g = dn_gs[sti]
                rp_g = rpgp.tile([P, 1024], bf, tag="rpg",
                                 name=f"rpg{sti}")
                with nc.allow_low_precision(
                        reason="1/denom rounds to bf16 for the broadcast"):
                    nc.vector.reciprocal(rp_g[0:8, :], dn_g[0:8, :])
                if _DEBUG:
                    nc.sync.dma_start(dbg_dn[sti * 8:sti * 8 + 8, :],
                                      dn_g[0:8, :])
                for hp in range(8):
                    zraw = zraws[hp]
                    # one wide broadcast covers both parities (cols 0:512
                    # even, 512:1024 odd); psum borrowed from the sp pool
                    for par in range(2):
                        bc = ps512.tile([P, 512], f32, tag="b",
                                        name=f"bc{hp}_{sti}_{par}")
                        nc.tensor.matmul(
                            bc[0:64, 0:W],
                            selc[0:8, hp * 64:(hp + 1) * 64],
                            rp_g[0:8, 512 * par:512 * par + W],
                            start=True, stop=True)
                        with nc.allow_low_precision(reason="bf16 yT"):
                            if par == 0:
                                nc.vector.tensor_tensor(
                                    yT[0:64, hp, s0:s0 + W],
                                    zraw[0:64, 0:W], bc[0:64, 0:W],
                                    op=Alu.mult)
                            else:
                                # odd heads partition-shift 0:64 -> 64:128
                                # via SBUF->SBUF DMA
                                zt_o = ztp.tile([64, 512], bf, tag="zt",
                                                name=f"zt{hp}_{sti}")
                                nc.vector.tensor_tensor(
                                    zt_o[0:64, 0:W],
                                    zraw[0:64, 512:512 + W], bc[0:64, 0:W],
                                    op=Alu.mult)
                                nc.sync.dma_start(
                                    yT[64:P, hp, s0:s0 + W],
                                    zt_o[0:64, 0:W])

            # ---------------- output projection ----------------
            def emit_wp_loads():
                wps = {}
                for fi, (f0, Fw) in enumerate(FT):
                    for k in range(KT):
                        w = wpp.tile([P, 512], bf, tag="wp",
                                     name=f"wp{fi}_{k}")
                        nc.sync.dma_start(
                            w[0:P, 0:Fw], wp_d[k * P:(k + 1) * P, f0:f0 + Fw])
                        wps[(fi, k)] = w
                return wps

            def emit_p4(wps, fi, sb):
                f0, Fw = FT[fi]
                rows = LAST if sb == NB - 1 else P
                r0 = sb * P
                ps = ps512.tile([P, 512], f32, tag="b", name=f"p4{fi}_{sb}")
                for k in range(KT):
                    nc.tensor.matmul(
                        ps[0:rows, 0:Fw],
                        yT[:, k, r0:r0 + rows],
                        wps[(fi, k)][0:P, 0:Fw],
                        start=(k == 0), stop=False)
                # + bias row via K=1 ones matmul
                nc.tensor.matmul(
                    ps[0:rows, 0:Fw],
                    ones_b[0:1, 0:rows],
                    bp_sb[0:1, f0:f0 + Fw],
                    start=False, stop=True)
                with nc.allow_low_precision(reason="bf16 stage"):
                    nc.scalar.activation(
                        ostage[0:rows, sb, f0:f0 + Fw],
                        ps[0:rows, 0:Fw], AF.Copy)

            def emit_out(fi, sb):
                f0, Fw = FT[fi]
                rows = LAST if sb == NB - 1 else P
                r0 = sb * P
                ot = outp.tile([P, 512], f32, tag="ot", name=f"ot{fi}_{sb}")
                act = AF.Identity if _NO_GELU else AF.Gelu
                nc.scalar.activation(
                    ot[0:rows, 0:Fw], ostage[0:rows, sb, f0:f0 + Fw], act)
                nc.sync.dma_start(out_d[r0:r0 + rows, f0:f0 + Fw],
                                  ot[0:rows, 0:Fw])

            # ---------------- interleaved schedule ----------------
            # QKV chunks keep the PE fed while earlier head-pairs run
            # their (ACT/DVE-heavy) attention; once st0 is complete for
            # all head-pairs, output-projection chunks fill the PE during
            # st1 attention. GELU runs once at the very end.
            z0, z1 = {}, {}
            emit_dng(0)
            emit_qk(0)
            emit_v(0)
            emit_qk(1)
            z0[0] = emit_attn(0, 0)
            emit_qk(2)
            z0[1] = emit_attn(1, 0)
            emit_qk(3)
            z0[2] = emit_attn(2, 0)
            emit_v(1)
            z0[3] = emit_attn(3, 0)
            z0[4] = emit_attn(4, 0)
            z0[5] = emit_attn(5, 0)
            z0[6] = emit_attn(6, 0)
            z0[7] = emit_attn(7, 0)
            wps = emit_wp_loads()
            emit_dng(1)
            # A(0,1)+A(1,1) keep the PE busy while norm(0) runs on DVE
            z1[0] = emit_attn(0, 1)
            z1[1] = emit_attn(1, 1)
            emit_norm(0, z0)
            # p4 for s-blocks 0-3 only needs st0 results; sb 4-7 need
            # every head-pair's st1, so they trail the last attention.
            emit_p4(wps, 0, 0)
            emit_p4(wps, 1, 0)
            z1[2] = emit_attn(2, 1)
            emit_p4(wps, 0, 1)
            z1[3] = emit_attn(3, 1)
            emit_p4(wps, 1, 1)
            z1[4] = emit_attn(4, 1)
            emit_p4(wps, 0, 2)
            z1[5] = emit_attn(5, 1)
            emit_p4(wps, 1, 2)
            z1[6] = emit_attn(6, 1)
            emit_p4(wps, 0, 3)
            z1[7] = emit_attn(7, 1)
            emit_p4(wps, 1, 3)
            emit_norm(1, z1)
            # sb 4-7 stores can only become ready after the last attention
            # EXP (their p4 chunks follow norm(1)), so the scheduler cannot
            # hoist their GELUs into the Exp stretch: emit normally and they
            # pipeline with the trailing p4 chunks.
            for sb in range(4, NB):
                emit_p4(wps, 0, sb)
                emit_out(0, sb)
                emit_p4(wps, 1, sb)
                emit_out(1, sb)
            # sb 0-3 staging is ready mid-attention; force these GELUs to
            # the end so no Exp<->Gelu activation-table swap (1.3us each)
            # lands inside the attention stretch.
            with tc.tile_wait_until(0.5):
                for sb in range(4):
                    emit_out(0, sb)
                    emit_out(1, sb)

            if _DEBUG:
                nc.sync.dma_start(dbg_yT[:, :, :], yT[:, :, :])
                nc.sync.dma_start(dbg_qT[:, :, :], qT[:, :, :])
                nc.sync.dma_start(dbg_kT[:, :, :], kT[:, :, :])

    nc.compile()
    return nc


def get_nc():
    if "nc" not in _CACHE:
        _CACHE["nc"] = _build_nc()
    return _CACHE["nc"]


def make_in_maps(inputs):
    import ml_dtypes
    bfnp = ml_dtypes.bfloat16

    x = np.asarray(inputs["x"], np.float32)
    wq = np.asarray(inputs["wq"], np.float32)
    wk = np.asarray(inputs["wk"], np.float32)
    wv = np.asarray(inputs["wv"], np.float32)
    wp = np.asarray(inputs["wp"], np.float32)
    bq = np.asarray(inputs["bq"], np.float32)
    bk = np.asarray(inputs["bk"], np.float32)
    bv = np.asarray(inputs["bv"], np.float32)
    bp = np.asarray(inputs["bp"], np.float32)

    # [H, E, D] -> [E, H*D] (concat head outputs along columns)
    wq2 = np.ascontiguousarray(
        wq.transpose(1, 0, 2).reshape(E, E).astype(bfnp))
    wk2 = np.ascontiguousarray(
        wk.transpose(1, 0, 2).reshape(E, E).astype(bfnp))
    wv2 = np.ascontiguousarray(
        wv.transpose(1, 0, 2).reshape(E, E).astype(bfnp))
    wp2 = np.ascontiguousarray(wp.astype(bfnp))
    # per-partition bias layout: bqt[p, m] = bq_flat[m*128 + p]
    bqt = np.ascontiguousarray(bq.reshape(-1).reshape(KT, P).T)
    bkt = np.ascontiguousarray(bk.reshape(-1).reshape(KT, P).T)
    # fold bv into output bias: y = z + bv  =>  out += bv @ wp
    bpe = (bp.astype(np.float64)
           + bv.reshape(-1).astype(np.float64) @ wp.astype(np.float64))
    bpe = np.ascontiguousarray(
        bpe.astype(np.float32).reshape(1, E).astype(bfnp))

    shared = {"wq2": wq2, "wk2": wk2, "wv2": wv2, "wp2": wp2,
              "bqt": bqt, "bkt": bkt, "bpe": bpe}
    return [dict(shared, xb=np.ascontiguousarray(x[b].astype(bfnp)))
            for b in range(B)]


def run(inputs, trace=False):
    from concourse.bass_utils import run_bass_kernel_spmd
    nc = get_nc()
    in_maps = make_in_maps(inputs)
    res = run_bass_kernel_spmd(nc, in_maps, list(range(NCORES)), trace=trace)
    out = np.stack([np.asarray(res.results[i]["out"]) for i in range(NCORES)])
    return out.astype(np.float32), res


def kernel(**inputs):
    out, _ = run(inputs, trace=False)
    return out
